# revision 15
# baseline (speedup 1.0000x reference)
"""CQT generator kernel for Trainium2 (8 NeuronCores, SPMD).

Math: for bin k (84 bins), out[k, t] = |sum_n xpad_k[t*512 + n] * kern_k[n]|
where kern_k is a hann-windowed DFT-bin kernel of length wl_k (111..13453)
and xpad_k is the waveform reflect-padded by wl_k//2.

Formulation used here: all bins' kernels are zero-extended into a shared
global window [0, 2P) (P = 6784, multiple of 128) centered so that
out[k, t] = sum_u xg[t*512 + u] * Kpad_k[u], with xg the globally
reflect-padded signal. Kernel supports are nested intervals (window length
is strictly decreasing in k), so for each 128-sample chunk c of u the set
of active bins is a prefix {0..m_c-1}.

Device program (per core, frames split 8 x 625):
  - rhs: the core's signal slice, stored 4-phase deinterleaved as
    X4[m, p, j] = slice[(4*j+p)*128 + m]; chunk c of frame tile [f0, f0+N)
    is the contiguous columns X4[:, c%4, f0+c//4 : f0+c//4+N].
  - matmul per chunk: lhsT = packed kernel block (128, M_c) with columns
    [r_0, i_0, r_1, i_1, ...]; accumulated into PSUM tile (prefix rows).
    Bins 0..63 -> tile A (128 rows), bins 64..83 -> tile B (40 rows).
  - magnitude: square (ACT) -> pair-sum via small matmul -> sqrt (ACT).
"""

import numpy as np
import ml_dtypes

import concourse.bass as bass
import concourse.mybir as mybir
import concourse.tile as tile
from concourse import bacc
from concourse import bass_utils

# ---- problem constants (hardcoded; must match the reference) ----
SR = 16000
HOP = 512
FMIN = 20.0
N_BINS = 84
BPO = 12
Q = 1.0 / (2.0 ** (1.0 / BPO) - 1.0)
L = 2560000
NUM_FRAMES = (L - HOP) // HOP + 1  # 5000
N_CORES = 8
T_CORE = NUM_FRAMES // N_CORES  # 625
P_PAD = 6784  # global half-window, multiple of 128, >= 6727
N_CHUNKS = 2 * P_PAD // 128  # 106
A_BINS = 64
B_ROWS = 2 * (N_BINS - A_BINS)  # 40
# fp32r matmuls require even free-dim counts and run at full rate only for
# free dim >= 256: compute 626 frames per core (tiles 314 + 312), host
# discards the 626th frame.
T_COMP = T_CORE + 1  # 626
NEED = (T_COMP - 1) * HOP + 2 * P_PAD  # samples needed per core slice
J4 = -(-(-(-NEED // 128)) // 4)  # ceil(ceil(NEED/128)/4) = 652
FRAME_TILES = [(0, 314), (314, 312)]

MM_DT = mybir.dt.float32r
MM_NP = np.float32


def _bin_params():
    out = []
    for k in range(N_BINS):
        freq = FMIN * 2.0 ** (k / BPO)
        wl = min(int(SR * Q / freq), L // 4)
        pad = wl // 2
        n = np.arange(wl)
        w = 0.5 - 0.5 * np.cos(2.0 * np.pi * n / wl)
        fb = min(int(freq * wl / SR), wl // 2)
        ang = 2.0 * np.pi * fb * n / wl
        kr = (w * np.cos(ang)).astype(np.float32)
        ki = (-w * np.sin(ang)).astype(np.float32)
        out.append((wl, pad, kr, ki))
    return out


def _build_plan():
    bins = _bin_params()
    starts = np.array([P_PAD - b[1] for b in bins])
    ends = np.array([P_PAD - b[1] + b[0] for b in bins])
    m_c = np.array([
        int(np.count_nonzero((starts < (c + 1) * 128) & (ends > c * 128)))
        for c in range(N_CHUNKS)
    ])
    MA = 2 * np.minimum(m_c, A_BINS)
    MB = 2 * np.maximum(0, m_c - A_BINS)
    # fp32r PSUM accumulation requires that the first (start=True) matmul of
    # a group covers every row later matmuls touch (the overwrite-on-clear
    # path miscomputes for fp32r). So issue a full-M chunk first; KA/KB
    # columns are laid out in issue order so split DMAs unblock the PE
    # progressively.
    cA_full = int(np.argmax(MA))
    cB_list = [c for c in range(N_CHUNKS) if MB[c] > 0]
    cB_full = cB_list[int(np.argmax(MB[cB_list]))]
    orderA = [cA_full] + [c for c in range(N_CHUNKS) if c != cA_full]
    orderB = [cB_full] + [c for c in cB_list if c != cB_full]
    offA = np.zeros(N_CHUNKS + 1, np.int64)
    offB = np.zeros(N_CHUNKS + 1, np.int64)
    accA = np.concatenate([[0], np.cumsum([MA[c] for c in orderA])])
    for pos, c in enumerate(orderA):
        offA[c] = accA[pos]
    accB = np.concatenate([[0], np.cumsum([MB[c] for c in orderB])])
    for pos, c in enumerate(orderB):
        offB[c] = accB[pos]
    KA = np.zeros((128, int(accA[-1])), np.float32)
    KB = np.zeros((128, int(accB[-1])), np.float32)
    for c in range(N_CHUNKS):
        u0 = c * 128
        for j in range(m_c[c]):
            wl, pad, kr, ki = bins[j]
            n0 = u0 - (P_PAD - pad)
            lo, hi = max(0, -n0), min(128, wl - n0)
            if j < A_BINS:
                col = offA[c] + 2 * j
                KA[lo:hi, col] = kr[n0 + lo : n0 + hi]
                KA[lo:hi, col + 1] = ki[n0 + lo : n0 + hi]
            else:
                col = offB[c] + 2 * (j - A_BINS)
                KB[lo:hi, col] = kr[n0 + lo : n0 + hi]
                KB[lo:hi, col + 1] = ki[n0 + lo : n0 + hi]
    pairA = np.zeros((128, A_BINS), np.float32)
    pairA[2 * np.arange(A_BINS), np.arange(A_BINS)] = 1
    pairA[2 * np.arange(A_BINS) + 1, np.arange(A_BINS)] = 1
    nb = N_BINS - A_BINS
    pairB = np.zeros((B_ROWS, nb), np.float32)
    pairB[2 * np.arange(nb), np.arange(nb)] = 1
    pairB[2 * np.arange(nb) + 1, np.arange(nb)] = 1
    return dict(MA=MA, MB=MB, offA=offA, offB=offB, KA=KA, KB=KB,
                orderA=orderA, orderB=orderB, pairA=pairA, pairB=pairB)


def _build_program(plan, reps=1):
    MA, MB, offA, offB = plan["MA"], plan["MB"], plan["offA"], plan["offB"]
    CA, CB = plan["KA"].shape[1], plan["KB"].shape[1]
    f32 = mybir.dt.float32

    nc = bacc.Bacc("TRN2", target_bir_lowering=False, debug=False,
                   num_devices=N_CORES)
    x4_d = nc.dram_tensor("x4", (128, 4, J4), MM_DT, kind="ExternalInput")
    ka_d = nc.dram_tensor("ka", (128, CA), MM_DT, kind="ExternalInput")
    kb_d = nc.dram_tensor("kb", (128, CB), MM_DT, kind="ExternalInput")
    pa_d = nc.dram_tensor("pa", (128, A_BINS), f32, kind="ExternalInput")
    pb_d = nc.dram_tensor("pb", (B_ROWS, N_BINS - A_BINS), f32,
                          kind="ExternalInput")
    out_d = nc.dram_tensor("out", (N_BINS, T_COMP), f32, kind="ExternalOutput")

    with tile.TileContext(nc) as tc:
        with (
            tc.tile_pool(name="const", bufs=1) as cpool,
            tc.tile_pool(name="work", bufs=2) as wpool,
            tc.tile_pool(name="psum", bufs=2, space="PSUM") as ppool,
        ):
            for _rep in range(reps):
                xs = cpool.tile([128, 4, J4], MM_DT, tag="xs")
                nc.sync.dma_start(xs[:], x4_d[:])
                kas = cpool.tile([128, CA], MM_DT, tag="kas")
                # split into column pieces so early chunks' matmuls can
                # start before the whole filterbank has loaded
                n_pieces = 4
                cuts = [CA * i // n_pieces for i in range(n_pieces + 1)]
                for i in range(n_pieces):
                    nc.sync.dma_start(
                        kas[:, cuts[i] : cuts[i + 1]],
                        ka_d[:, cuts[i] : cuts[i + 1]],
                    )
                kbs = cpool.tile([128, CB], MM_DT, tag="kbs")
                nc.sync.dma_start(kbs[:], kb_d[:])
                pas = cpool.tile([128, A_BINS], f32, tag="pas")
                nc.sync.dma_start(pas[:], pa_d[:])
                pbs = cpool.tile([B_ROWS, N_BINS - A_BINS], f32, tag="pbs")
                nc.sync.dma_start(pbs[:], pb_d[:])

                psums = []
                for f0, N in FRAME_TILES:
                    psA = ppool.tile([128, N], f32, tag="psA")
                    for idx, c in enumerate(plan["orderA"]):
                        nc.tensor.matmul(
                            psA[0 : MA[c], :],
                            kas[:, offA[c] : offA[c] + MA[c]],
                            xs[:, c % 4, f0 + c // 4 : f0 + c // 4 + N],
                            start=(idx == 0),
                            stop=(idx == len(plan["orderA"]) - 1),
                        )
                    psB = ppool.tile([B_ROWS, N], f32, tag="psB")
                    for idx, c in enumerate(plan["orderB"]):
                        nc.tensor.matmul(
                            psB[0 : MB[c], :],
                            kbs[:, offB[c] : offB[c] + MB[c]],
                            xs[:, c % 4, f0 + c // 4 : f0 + c // 4 + N],
                            start=(idx == 0),
                            stop=(idx == len(plan["orderB"]) - 1),
                        )
                    psums.append((psA, psB))

                for (f0, N), (psA, psB) in zip(FRAME_TILES, psums):
                    sqA = wpool.tile([128, N], f32, tag="sqA")
                    nc.scalar.square(sqA[:], psA[:])
                    sqB = wpool.tile([B_ROWS, N], f32, tag="sqB")
                    nc.scalar.square(sqB[:], psB[:])
                    psC = ppool.tile([A_BINS, N], f32, tag="psC")
                    nc.tensor.matmul(psC[:], pas[:], sqA[:])
                    psD = ppool.tile([N_BINS - A_BINS, N], f32, tag="psD")
                    nc.tensor.matmul(psD[:], pbs[:], sqB[:])
                    ot = wpool.tile([N_BINS, N], f32, tag="ot")
                    nc.scalar.sqrt(ot[0:A_BINS, :], psC[:])
                    nc.scalar.sqrt(ot[A_BINS:N_BINS, :], psD[:])
                    nc.sync.dma_start(out_d[:, f0 : f0 + N], ot[:])

    nc.compile()
    return nc


def _make_runner(nc):
    """Build a cached jitted SPMD runner for a compiled Bass module.

    Mirrors concourse.bass2jax.run_bass_via_pjrt but keeps the jitted
    callable so repeated kernel() calls don't re-trace/re-compile.
    """
    import jax
    from jax.experimental.shard_map import shard_map
    from jax.sharding import Mesh, PartitionSpec
    from concourse import bass2jax
    from concourse import mybir as _mybir

    bass2jax.install_neuronx_cc_hook()

    partition_name = (
        nc.partition_id_tensor.name if nc.partition_id_tensor else None
    )
    in_names, out_names, out_avals = [], [], []
    for alloc in nc.m.functions[0].allocations:
        if not isinstance(_mybir.MemoryLocationSet, type) or not isinstance(
            alloc, _mybir.MemoryLocationSet
        ):
            continue
        name = alloc.memorylocations[0].name
        if alloc.kind == "ExternalInput":
            if name != partition_name:
                in_names.append(name)
        elif alloc.kind == "ExternalOutput":
            out_names.append(name)
            out_avals.append(
                jax.core.ShapedArray(
                    tuple(alloc.tensor_shape), _mybir.dt.np(alloc.dtype)
                )
            )
    n_params = len(in_names)
    n_outs = len(out_avals)
    all_in_names = in_names + out_names
    if partition_name is not None:
        all_in_names = all_in_names + [partition_name]

    def _body(*args):
        operands = list(args)
        if partition_name is not None:
            operands.append(bass2jax.partition_id_tensor())
        outs = bass2jax._bass_exec_p.bind(
            *operands,
            out_avals=tuple(out_avals),
            in_names=tuple(all_in_names),
            out_names=tuple(out_names),
            lowering_input_output_aliases=(),
            sim_require_finite=True,
            sim_require_nnan=True,
            nc=nc,
        )
        return tuple(outs)

    devices = jax.devices()[:N_CORES]
    mesh = Mesh(np.asarray(devices), ("core",))
    specs = (PartitionSpec("core"),) * (n_params + n_outs)
    donate = tuple(range(n_params, n_params + n_outs))
    sharded = jax.jit(
        shard_map(
            _body, mesh=mesh, in_specs=specs,
            out_specs=(PartitionSpec("core"),) * n_outs, check_rep=False,
        ),
        donate_argnums=donate,
        keep_unused=True,
    )

    def run(in_maps):
        concat_in = [
            np.concatenate([np.asarray(m[nm]) for m in in_maps], axis=0)
            for nm in in_names
        ]
        concat_zeros = [
            np.zeros((N_CORES * a.shape[0], *a.shape[1:]), a.dtype)
            for a in out_avals
        ]
        out_arrs = sharded(*concat_in, *concat_zeros)
        return [
            {
                nm: np.asarray(out_arrs[i]).reshape(
                    N_CORES, *out_avals[i].shape
                )[c]
                for i, nm in enumerate(out_names)
            }
            for c in range(N_CORES)
        ]

    run.sharded = sharded
    run.in_names = in_names
    run.out_names = out_names
    run.out_avals = out_avals
    return run


_CACHE = {}


def _get_compiled():
    if "nc" not in _CACHE:
        plan = _build_plan()
        _CACHE["plan"] = plan
        _CACHE["nc"] = _build_program(plan)
        _CACHE["runner"] = _make_runner(_CACHE["nc"])
        _CACHE["consts"] = {
            "ka": plan["KA"].astype(MM_NP),
            "kb": plan["KB"].astype(MM_NP),
            "pa": plan["pairA"],
            "pb": plan["pairB"],
        }
    return _CACHE["nc"], _CACHE["consts"]


def _make_in_maps(x):
    _, consts = _get_compiled()
    xg = np.pad(np.asarray(x, np.float32), P_PAD, mode="reflect")
    in_maps = []
    for d in range(N_CORES):
        base = d * T_CORE * HOP
        sl = xg[base : base + J4 * 512]
        buf = np.zeros(J4 * 512, np.float32)
        buf[: len(sl)] = sl
        x4 = np.ascontiguousarray(
            buf.reshape(J4, 4, 128).transpose(2, 1, 0)
        ).astype(MM_NP)
        in_maps.append({"x4": x4, **consts})
    return in_maps


def kernel(waveform):
    _get_compiled()
    in_maps = _make_in_maps(waveform)
    results = _CACHE["runner"](in_maps)
    out = np.concatenate([r["out"][:, :T_CORE] for r in results], axis=1)
    return out[None].astype(np.float32)


if __name__ == "__main__":
    x = np.random.default_rng(0).standard_normal(L).astype(np.float32)
    y = kernel(waveform=x)
    print("out:", y.shape, y.dtype, float(y.mean()), float(y.max()))


# revision 22
# speedup vs baseline: 49.0438x; 49.0438x over previous
"""CQT generator kernel for Trainium2 (8 NeuronCores, SPMD).

Math: for bin k (84 bins), out[k, t] = |sum_n xpad_k[t*512 + n] * kern_k[n]|
where kern_k is a hann-windowed DFT-bin kernel of length wl_k (111..13453)
and xpad_k is the waveform reflect-padded by wl_k//2.

Formulation: all bins' kernels are zero-extended into a shared global
window [0, 2P) (P = 6784, multiple of 128) centered so that
out[k, t] = sum_u xg[t*512 + u] * Kpad_k[u], with xg the globally
reflect-padded signal. Kernel supports are nested intervals (window length
strictly decreasing in k), so for each 128-sample chunk c of u the active
bins form a prefix {0..m_c-1}.

Device program (per core, frames split 8 x 625, computed as 626):
  - rhs: the core's signal slice, 4-phase deinterleaved:
    X4[m, p, j] = slice[(4*j+p)*128 + m]; chunk c of frame tile [f0, f0+N)
    is the contiguous columns X4[:, c%4, f0+c//4 : f0+c//4+N].
  - per chunk c: matmul lhsT = packed kernel block (128, M_c), columns
    [r_0, i_0, r_1, i_1, ...] (canonical rows), accumulated into PSUM.
    Bins 0..63 -> "A" rows (128), bins 64..83 -> "B" rows (40, own bank).
  - tiered col-group packing (bf16 mode): chunks with M>64 run serial at
    full width into psBig (canonical rows); 32<M<=64 run 2-concurrent in
    64-wide col groups (copies of canonical rows at partition 64g+r);
    M<=32 run 4-concurrent in 32-wide groups (copies at 32g+r). Copy
    banks are reduced with 0/1 matmuls (R64/R32) accumulated into psBig.
    A full-width zero-weight dummy matmul initializes each copy bank
    (has_written) so concurrent group matmuls can all accumulate.
  - magnitude: ACT square (scale 0.5) -> pair-sum matmul -> sqrt(scale 4).
"""

import numpy as np
import ml_dtypes

import concourse.bass as bass
import concourse.mybir as mybir
import concourse.tile as tile
from concourse import bacc
from concourse import bass_utils

# ---- problem constants (must match the reference) ----
SR = 16000
HOP = 512
FMIN = 20.0
N_BINS = 84
BPO = 12
Q = 1.0 / (2.0 ** (1.0 / BPO) - 1.0)
L = 2560000
NUM_FRAMES = (L - HOP) // HOP + 1  # 5000
N_CORES = 8
T_CORE = NUM_FRAMES // N_CORES  # 625
P_PAD = 6784  # global half-window, multiple of 128, >= 6727
N_CHUNKS = 2 * P_PAD // 128  # 106
A_BINS = 64
B_ROWS = 2 * (N_BINS - A_BINS)  # 40

# frames are computed as 626 per core (even tiles); host discards the last
T_COMP = T_CORE + 1  # 626
NEED = (T_COMP - 1) * HOP + 2 * P_PAD
J4 = -(-(-(-NEED // 128)) // 4)  # 652

MODE = "tiered_bf16"  # or "flat_f32r"

if MODE == "tiered_bf16":
    MM_DT = mybir.dt.bfloat16
    MM_NP = ml_dtypes.bfloat16
    FRAME_TILES = [(0, 314), (314, 312)]
else:
    MM_DT = mybir.dt.float32r
    MM_NP = np.float32
    FRAME_TILES = [(0, 314), (314, 312)]

F16_DT = mybir.dt.float16
F16_NP = np.float16


def _bin_params():
    out = []
    for k in range(N_BINS):
        freq = FMIN * 2.0 ** (k / BPO)
        wl = min(int(SR * Q / freq), L // 4)
        pad = wl // 2
        n = np.arange(wl)
        w = 0.5 - 0.5 * np.cos(2.0 * np.pi * n / wl)
        fb = min(int(freq * wl / SR), wl // 2)
        ang = 2.0 * np.pi * fb * n / wl
        kr = (w * np.cos(ang)).astype(np.float32)
        ki = (-w * np.sin(ang)).astype(np.float32)
        out.append((wl, pad, kr, ki))
    return out


def _chunk_block(bins, c, j_lo, j_hi, width):
    """Kernel block for chunk c, bins [j_lo, j_hi), zero-padded to width
    columns. Column 2*(j-j_lo)(+1) = real/imag of bin j."""
    blk = np.zeros((128, width), np.float32)
    u0 = c * 128
    for j in range(j_lo, j_hi):
        wl, pad, kr, ki = bins[j]
        n0 = u0 - (P_PAD - pad)
        lo, hi = max(0, -n0), min(128, wl - n0)
        if hi <= lo:
            continue
        col = 2 * (j - j_lo)
        blk[lo:hi, col] = kr[n0 + lo : n0 + hi]
        blk[lo:hi, col + 1] = ki[n0 + lo : n0 + hi]
    return blk


def _build_plan():
    bins = _bin_params()
    starts = np.array([P_PAD - b[1] for b in bins])
    ends = np.array([P_PAD - b[1] + b[0] for b in bins])
    m_c = np.array([
        int(np.count_nonzero((starts < (c + 1) * 128) & (ends > c * 128)))
        for c in range(N_CHUNKS)
    ])
    MA = 2 * np.minimum(m_c, A_BINS)
    MB = 2 * np.maximum(0, m_c - A_BINS)

    plan = dict(bins=bins, m_c=m_c, MA=MA, MB=MB)

    # ---- B side (bins 64..83): few central chunks, serial ----
    cB_list = [c for c in range(N_CHUNKS) if MB[c] > 0]
    cB_full = cB_list[int(np.argmax(MB[cB_list]))]
    orderB = [cB_full] + [c for c in cB_list if c != cB_full]
    offB, colB = {}, 0
    for c in orderB:
        offB[c] = colB
        colB += int(MB[c])
    KB = np.zeros((128, colB), np.float32)
    for c in orderB:
        KB[:, offB[c] : offB[c] + MB[c]] = _chunk_block(
            bins, c, A_BINS, A_BINS + MB[c] // 2, int(MB[c])
        )
    plan.update(orderB=orderB, offB=offB, KB=KB)

    nb = N_BINS - A_BINS
    pairB = np.zeros((B_ROWS, nb), np.float32)
    pairB[2 * np.arange(nb), np.arange(nb)] = 1
    pairB[2 * np.arange(nb) + 1, np.arange(nb)] = 1
    pairA = np.zeros((128, A_BINS), np.float32)
    pairA[2 * np.arange(A_BINS), np.arange(A_BINS)] = 1
    pairA[2 * np.arange(A_BINS) + 1, np.arange(A_BINS)] = 1
    plan.update(pairA=pairA, pairB=pairB)

    if MODE == "flat_f32r":
        # full-M-first ordering (fp32r overwrite-on-clear bug workaround)
        cA_full = int(np.argmax(MA))
        orderA = [cA_full] + [c for c in range(N_CHUNKS) if c != cA_full]
        offA, colA = {}, 0
        for c in orderA:
            offA[c] = colA
            colA += int(MA[c])
        KA = np.zeros((128, colA), np.float32)
        for c in orderA:
            KA[:, offA[c] : offA[c] + MA[c]] = _chunk_block(
                bins, c, 0, min(m_c[c], A_BINS), int(MA[c])
            )
        plan.update(orderA=orderA, offA=offA, KA=KA)
        return plan

    # ---- tiered bf16 ----
    big = [c for c in range(N_CHUNKS) if MA[c] > 64]
    med = [c for c in range(N_CHUNKS) if 32 < MA[c] <= 64]
    sml = [c for c in range(N_CHUNKS) if MA[c] <= 32]
    # big: a full-width (M=128) chunk first
    cfull = big[int(np.argmax(MA[big]))]
    assert MA[cfull] == 128
    orderBig = [cfull] + [c for c in big if c != cfull]
    offBig, colBig = {}, 0
    for c in orderBig:
        offBig[c] = colBig
        colBig += int(MA[c])
    KBIG = np.zeros((128, colBig), np.float32)
    for c in orderBig:
        KBIG[:, offBig[c] : offBig[c] + MA[c]] = _chunk_block(
            bins, c, 0, min(m_c[c], A_BINS), int(MA[c])
        )
    # med: 64-wide blocks, groups alternate 0/1 (base partition 64*g)
    medAsg = [(c, i % 2) for i, c in enumerate(med)]
    KMED = np.zeros((128, 64 * len(med)), np.float32)
    for i, c in enumerate(med):
        KMED[:, 64 * i : 64 * (i + 1)] = _chunk_block(
            bins, c, 0, min(m_c[c], A_BINS), 64
        )
    # small: 32-wide blocks, groups cycle 0..3 (base 32*g)
    smlAsg = [(c, i % 4) for i, c in enumerate(sml)]
    KSML = np.zeros((128, 32 * len(sml)), np.float32)
    for i, c in enumerate(sml):
        KSML[:, 32 * i : 32 * (i + 1)] = _chunk_block(
            bins, c, 0, min(m_c[c], A_BINS), 32
        )
    # reduction matrices (f16-exact 0/1)
    R64 = np.zeros((128, 64), np.float32)
    R64[np.arange(128), np.arange(128) % 64] = 1
    R32 = np.zeros((128, 32), np.float32)
    R32[np.arange(128), np.arange(128) % 32] = 1
    plan.update(orderBig=orderBig, offBig=offBig, KBIG=KBIG,
                medAsg=medAsg, KMED=KMED, smlAsg=smlAsg, KSML=KSML,
                R64=R64, R32=R32)
    return plan


def _rhs(xs, c, f0, N):
    return xs[:, c % 4, f0 + c // 4 : f0 + c // 4 + N]


def _build_program(plan, reps=1, loop_n=None):
    f32 = mybir.dt.float32
    MA, MB = plan["MA"], plan["MB"]

    nc = bacc.Bacc("TRN2", target_bir_lowering=False, debug=False,
                   num_devices=N_CORES)
    x4_d = nc.dram_tensor("x4", (128, 4, J4), MM_DT, kind="ExternalInput")
    kb_d = nc.dram_tensor("kb", (128, plan["KB"].shape[1]), MM_DT,
                          kind="ExternalInput")
    pa_d = nc.dram_tensor("pa", (128, A_BINS), F16_DT, kind="ExternalInput")
    pb_d = nc.dram_tensor("pb", (B_ROWS, N_BINS - A_BINS), F16_DT,
                          kind="ExternalInput")
    out_d = nc.dram_tensor("out", (N_BINS, T_COMP), f32, kind="ExternalOutput")
    if MODE == "tiered_bf16":
        kbig_d = nc.dram_tensor("kbig", (128, plan["KBIG"].shape[1]), MM_DT,
                                kind="ExternalInput")
        kmed_d = nc.dram_tensor("kmed", (128, plan["KMED"].shape[1]), MM_DT,
                                kind="ExternalInput")
        ksml_d = nc.dram_tensor("ksml", (128, plan["KSML"].shape[1]), MM_DT,
                                kind="ExternalInput")
        r64_d = nc.dram_tensor("r64", (128, 64), F16_DT, kind="ExternalInput")
        r32_d = nc.dram_tensor("r32", (128, 32), F16_DT, kind="ExternalInput")
    else:
        ka_d = nc.dram_tensor("ka", (128, plan["KA"].shape[1]), MM_DT,
                              kind="ExternalInput")

    with tile.TileContext(nc) as tc:
        with (
            tc.tile_pool(name="const", bufs=2) as cpool,
            tc.tile_pool(name="work", bufs=2) as wpool,
            tc.tile_pool(name="psum", bufs=1, space="PSUM") as ppool,
        ):
            def body():
                xs = cpool.tile([128, 4, J4], MM_DT, tag="xs")
                nc.sync.dma_start(xs[:], x4_d[:])
                kbs = cpool.tile([128, plan["KB"].shape[1]], MM_DT, tag="kbs")
                nc.sync.dma_start(kbs[:], kb_d[:])
                pas = cpool.tile([128, A_BINS], F16_DT, tag="pas")
                nc.sync.dma_start(pas[:], pa_d[:])
                pbs = cpool.tile([B_ROWS, N_BINS - A_BINS], F16_DT, tag="pbs")
                nc.sync.dma_start(pbs[:], pb_d[:])
                if MODE == "tiered_bf16":
                    kbig = cpool.tile([128, plan["KBIG"].shape[1]], MM_DT,
                                      tag="kbig")
                    kmed = cpool.tile([128, plan["KMED"].shape[1]], MM_DT,
                                      tag="kmed")
                    ksml = cpool.tile([128, plan["KSML"].shape[1]], MM_DT,
                                      tag="ksml")
                    for t_d, t_s in ((kbig_d, kbig), (kmed_d, kmed),
                                     (ksml_d, ksml)):
                        CW = t_s.shape[-1]
                        half = (CW // 2) & ~1
                        nc.sync.dma_start(t_s[:, :half], t_d[:, :half])
                        nc.sync.dma_start(t_s[:, half:], t_d[:, half:])
                    r64s = cpool.tile([128, 64], F16_DT, tag="r64s")
                    nc.sync.dma_start(r64s[:], r64_d[:])
                    r32s = cpool.tile([128, 32], F16_DT, tag="r32s")
                    nc.sync.dma_start(r32s[:], r32_d[:])
                    zt = cpool.tile([128, 128], MM_DT, tag="zt")
                    nc.gpsimd.memset(zt[:], 0.0)
                else:
                    kas = cpool.tile([128, plan["KA"].shape[1]], MM_DT,
                                     tag="kas")
                    CW = plan["KA"].shape[1]
                    n_pieces = 4
                    cuts = [(CW * i // n_pieces) & ~1 for i in range(n_pieces)]
                    cuts.append(CW)
                    for i in range(n_pieces):
                        nc.sync.dma_start(kas[:, cuts[i] : cuts[i + 1]],
                                          ka_d[:, cuts[i] : cuts[i + 1]])

                for f0, N in FRAME_TILES:
                    if MODE == "tiered_bf16":
                        psBig = ppool.tile([128, N], f32, tag="psBig", bufs=2)
                        ordB_ = plan["orderBig"]
                        for idx, c in enumerate(ordB_):
                            nc.tensor.matmul(
                                psBig[0 : MA[c], :],
                                kbig[:, plan["offBig"][c] : plan["offBig"][c] + MA[c]],
                                _rhs(xs, c, f0, N),
                                start=(idx == 0), stop=False,
                            )
                        psMed = ppool.tile([128, N], f32, tag="psMed", bufs=1)
                        nc.tensor.matmul(psMed[:, 0:2], zt[:], xs[:, 0, f0 : f0 + 2],
                                         start=True, stop=False)
                        for i, (c, g) in enumerate(plan["medAsg"]):
                            nc.tensor.matmul(
                                psMed[64 * g : 64 * g + 64, :],
                                kmed[:, 64 * i : 64 * (i + 1)],
                                _rhs(xs, c, f0, N),
                                start=False, stop=(i == len(plan["medAsg"]) - 1),
                                tile_position=(0, 64 * g),
                            )
                        psSml = ppool.tile([128, N], f32, tag="psSml", bufs=1)
                        nc.tensor.matmul(psSml[:, 0:2], zt[:], xs[:, 0, f0 : f0 + 2],
                                         start=True, stop=False)
                        for i, (c, g) in enumerate(plan["smlAsg"]):
                            nc.tensor.matmul(
                                psSml[32 * g : 32 * g + 32, :],
                                ksml[:, 32 * i : 32 * (i + 1)],
                                _rhs(xs, c, f0, N),
                                start=False, stop=(i == len(plan["smlAsg"]) - 1),
                                tile_position=(0, 32 * g),
                            )
                    else:
                        psBig = ppool.tile([128, N], f32, tag="psBig", bufs=2)
                        ordA = plan["orderA"]
                        for idx, c in enumerate(ordA):
                            nc.tensor.matmul(
                                psBig[0 : MA[c], :],
                                kas[:, plan["offA"][c] : plan["offA"][c] + MA[c]],
                                _rhs(xs, c, f0, N),
                                start=(idx == 0), stop=(idx == len(ordA) - 1),
                            )
                        psMed = psSml = None
                    psB = ppool.tile([B_ROWS, N], f32, tag="psB", bufs=1)
                    for idx, c in enumerate(plan["orderB"]):
                        nc.tensor.matmul(
                            psB[0 : MB[c], :],
                            kbs[:, plan["offB"][c] : plan["offB"][c] + MB[c]],
                            _rhs(xs, c, f0, N),
                            start=(idx == 0),
                            stop=(idx == len(plan["orderB"]) - 1),
                        )

                    # epilogue (same tile): copy groups -> reduce -> magnitude
                    if MODE == "tiered_bf16":
                        mcp = wpool.tile([128, N], F16_DT, tag="mcp")
                        nc.vector.tensor_copy(mcp[:], psMed[:])
                        scp = wpool.tile([128, N], F16_DT, tag="scp")
                        nc.vector.tensor_copy(scp[:], psSml[:])
                        nc.tensor.matmul(psBig[0:64, :], r64s[:], mcp[:],
                                         start=False, stop=False)
                        nc.tensor.matmul(psBig[0:32, :], r32s[:], scp[:],
                                         start=False, stop=True)
                    # magnitude: square with 0.5 scale (f16 range), pair-sum,
                    # sqrt with 4.0 scale (exact powers of two)
                    sqA = wpool.tile([128, N], F16_DT, tag="sqA")
                    nc.scalar.activation(sqA[:], psBig[:],
                                         mybir.ActivationFunctionType.Square,
                                         scale=0.5)
                    sqB = wpool.tile([B_ROWS, N], F16_DT, tag="sqB")
                    nc.scalar.activation(sqB[:], psB[:],
                                         mybir.ActivationFunctionType.Square,
                                         scale=0.5)
                    psC = ppool.tile([A_BINS, N], f32, tag="psC", bufs=1)
                    nc.tensor.matmul(psC[:], pas[:], sqA[:])
                    psD = ppool.tile([N_BINS - A_BINS, N], f32, tag="psD",
                                     bufs=1)
                    nc.tensor.matmul(psD[:], pbs[:], sqB[:])
                    ot = wpool.tile([N_BINS, N], f32, tag="ot")
                    nc.scalar.activation(ot[0:A_BINS, :], psC[:],
                                         mybir.ActivationFunctionType.Sqrt,
                                         scale=4.0)
                    nc.scalar.activation(ot[A_BINS:N_BINS, :], psD[:],
                                         mybir.ActivationFunctionType.Sqrt,
                                         scale=4.0)
                    nc.sync.dma_start(out_d[:, f0 : f0 + N], ot[:])

            if loop_n is not None:
                with tc.For_i(0, loop_n, 1):
                    body()
            else:
                for _rep in range(reps):
                    body()

    nc.compile()
    return nc


def _make_runner(nc):
    """Cached jitted SPMD runner (mirrors bass2jax.run_bass_via_pjrt but
    keeps the jitted callable so repeated kernel() calls don't re-trace)."""
    import jax
    from jax.experimental.shard_map import shard_map
    from jax.sharding import Mesh, PartitionSpec
    from concourse import bass2jax
    from concourse import mybir as _mybir

    bass2jax.install_neuronx_cc_hook()

    partition_name = (
        nc.partition_id_tensor.name if nc.partition_id_tensor else None
    )
    in_names, out_names, out_avals = [], [], []
    for alloc in nc.m.functions[0].allocations:
        if not isinstance(alloc, _mybir.MemoryLocationSet):
            continue
        name = alloc.memorylocations[0].name
        if alloc.kind == "ExternalInput":
            if name != partition_name:
                in_names.append(name)
        elif alloc.kind == "ExternalOutput":
            out_names.append(name)
            out_avals.append(
                jax.core.ShapedArray(
                    tuple(alloc.tensor_shape), _mybir.dt.np(alloc.dtype)
                )
            )
    n_params = len(in_names)
    n_outs = len(out_avals)
    all_in_names = in_names + out_names
    if partition_name is not None:
        all_in_names = all_in_names + [partition_name]

    def _body(*args):
        operands = list(args)
        if partition_name is not None:
            operands.append(bass2jax.partition_id_tensor())
        outs = bass2jax._bass_exec_p.bind(
            *operands,
            out_avals=tuple(out_avals),
            in_names=tuple(all_in_names),
            out_names=tuple(out_names),
            lowering_input_output_aliases=(),
            sim_require_finite=True,
            sim_require_nnan=True,
            nc=nc,
        )
        return tuple(outs)

    devices = jax.devices()[:N_CORES]
    mesh = Mesh(np.asarray(devices), ("core",))
    specs = (PartitionSpec("core"),) * (n_params + n_outs)
    donate = tuple(range(n_params, n_params + n_outs))
    sharded = jax.jit(
        shard_map(
            _body, mesh=mesh, in_specs=specs,
            out_specs=(PartitionSpec("core"),) * n_outs, check_rep=False,
        ),
        donate_argnums=donate,
        keep_unused=True,
    )

    def run(in_maps):
        concat_in = [
            np.concatenate([np.asarray(m[nm]) for m in in_maps], axis=0)
            for nm in in_names
        ]
        concat_zeros = [
            np.zeros((N_CORES * a.shape[0], *a.shape[1:]), a.dtype)
            for a in out_avals
        ]
        out_arrs = sharded(*concat_in, *concat_zeros)
        return [
            {
                nm: np.asarray(out_arrs[i]).reshape(
                    N_CORES, *out_avals[i].shape
                )[c]
                for i, nm in enumerate(out_names)
            }
            for c in range(N_CORES)
        ]

    run.sharded = sharded
    run.in_names = in_names
    run.out_names = out_names
    run.out_avals = out_avals
    return run


_CACHE = {}


def _get_compiled():
    if "nc" not in _CACHE:
        plan = _build_plan()
        _CACHE["plan"] = plan
        _CACHE["nc"] = _build_program(plan)
        _CACHE["runner"] = _make_runner(_CACHE["nc"])
        consts = {
            "kb": plan["KB"].astype(MM_NP),
            "pa": plan["pairA"].astype(F16_NP),
            "pb": plan["pairB"].astype(F16_NP),
        }
        if MODE == "tiered_bf16":
            consts.update(
                kbig=plan["KBIG"].astype(MM_NP),
                kmed=plan["KMED"].astype(MM_NP),
                ksml=plan["KSML"].astype(MM_NP),
                r64=plan["R64"].astype(F16_NP),
                r32=plan["R32"].astype(F16_NP),
            )
        else:
            consts["ka"] = plan["KA"].astype(MM_NP)
        _CACHE["consts"] = consts
    return _CACHE["nc"], _CACHE["consts"]


def _make_in_maps(x):
    _, consts = _get_compiled()
    xg = np.pad(np.asarray(x, np.float32), P_PAD, mode="reflect")
    in_maps = []
    for d in range(N_CORES):
        base = d * T_CORE * HOP
        sl = xg[base : base + J4 * 512]
        buf = np.zeros(J4 * 512, np.float32)
        buf[: len(sl)] = sl
        x4 = np.ascontiguousarray(
            buf.reshape(J4, 4, 128).transpose(2, 1, 0)
        ).astype(MM_NP)
        in_maps.append({"x4": x4, **consts})
    return in_maps


def kernel(waveform):
    _get_compiled()
    in_maps = _make_in_maps(waveform)
    results = _CACHE["runner"](in_maps)
    out = np.concatenate([r["out"][:, :T_CORE] for r in results], axis=1)
    return out[None].astype(np.float32)


if __name__ == "__main__":
    x = np.random.default_rng(0).standard_normal(L).astype(np.float32)
    y = kernel(waveform=x)
    print("out:", y.shape, y.dtype, float(y.mean()), float(y.max()))
